# revision 9
# baseline (speedup 1.0000x reference)
"""Trainium2 Bass kernel for a single character-decoding step
(embedding lookup -> LSTM cell -> additive attention over encoder states ->
concat projection -> vocab logits).

Strategy
--------
Data-parallel over batch across 8 NeuronCores (4 batches per core); the small
decoder weights are replicated.  Key structural facts used:

* The additive-attention logit is ``enc @ w_enc + (h_new @ w_hid + b)``; the
  second term is constant across sequence positions, and masked positions are
  *replaced* by -1e9, so the softmax (and therefore the context vector and
  final output) is independent of ``h_new``/``w_hid``/``attn_b``.  Attention
  and the LSTM run concurrently on different engines.
* Batches are sorted by ``encoder_lens`` and assigned to cores round-robin so
  that slot j of every core has a similar length; the per-slot tile count
  ``T_j = ceil(max_len_in_slot / 128)`` is baked into the compiled program,
  so padding tiles beyond the longest length in a slot are never read from
  HBM nor computed on.  (The program is compiled per distinct ``T`` tuple and
  cached.)

Per-core dataflow:
  - scores: one fused multiply+row-sum (``scalar_tensor_tensor`` accum) per
    (128 x 1024) encoder tile on the vector engine
  - softmax: free-dim reduce_max, gpsimd partition_all_reduce, Exp with fused
    ``-max`` bias and fused free-dim accumulation, partition_all_reduce(add),
    reciprocal
  - context: PE matmuls with the exp-weight column as the stationary operand
    over the SBUF-resident encoder tiles, accumulated in PSUM, normalized by
    1/sum on evacuation
  - LSTM: z^T = W^T stacked [x;h] with features on partitions (bf16 weights,
    PSUM column-blocked), gate nonlinearities fused with bias on the scalar
    engine
  - concat/out projections: f32 PE matmuls; the context vector is transposed
    to feature-on-partition with PE transposes.
"""

import math
import sys

sys.path.insert(0, "/opt/trn_rl_repo")

import numpy as np
import ml_dtypes

import concourse.bacc as bacc
import concourse.tile as tile
from concourse import mybir, bass_isa
from concourse.bass_utils import run_bass_kernel_spmd
from concourse.masks import make_identity

B, S, H, CHAR, VOCAB = 32, 2048, 1024, 256, 64
NCORES, BPC, P, HC = 8, 4, 128, 8
KC = (CHAR + H) // P          # 10 contraction chunks for the LSTM matmul
CW_KC = 2 * H // P            # 16 contraction chunks for the concat matmul
NEG = np.float32(-1e9)

F32 = mybir.dt.float32
BF16 = mybir.dt.bfloat16
BF_NP = ml_dtypes.bfloat16

# smalls_f32 blob column layout
OFF_WENC = 0                   # (128, 1024) broadcast w_enc
OFF_BIAS = 1024                # 32 cols: LSTM bias chunk m -> col OFF_BIAS+m
OFF_CB = 1056                  # 8 cols: concat bias chunks
OFF_OB = 1064                  # 1 col (rows 0:64): out bias
OFF_CT = 1065                  # 32 cols: c_prev^T chunks (hc, b) -> 1065+hc*4+b
OFF_OWT = 1097                 # 512 cols: out_w^T chunks (kc, v) -> 1097+kc*64+v
OFF_MASK = 1609                # sum(T) cols of additive masks
# smalls_bf16 blob: 40 cols of [x; h]^T chunks (kc, b) -> kc*4+b


def _build(T):
    """Emit + compile the 8-core SPMD program for per-slot tile counts T."""
    sum_t = sum(T)
    w32 = OFF_MASK + sum_t
    nc = bacc.Bacc("TRN2", target_bir_lowering=False, debug=False,
                   num_devices=NCORES)

    sf_d = nc.dram_tensor("sf", [P, w32], F32, kind="ExternalInput")
    sb_d = nc.dram_tensor("sb", [P, KC * BPC], BF16, kind="ExternalInput")
    wl_d = nc.dram_tensor("wl", [KC, P, 4 * H], BF16, kind="ExternalInput")
    cw_d = nc.dram_tensor("cw", [CW_KC, P, H], F32, kind="ExternalInput")
    enc_d = [nc.dram_tensor(f"enc{j}", [T[j] * P, H], F32, kind="ExternalInput")
             for j in range(BPC)]
    lg_d = nc.dram_tensor("lg", [VOCAB, BPC], F32, kind="ExternalOutput")
    hc_d = nc.dram_tensor("hc", [P, 2 * HC * BPC], F32, kind="ExternalOutput")

    AF = mybir.ActivationFunctionType
    ALU = mybir.AluOpType

    with tile.TileContext(nc) as tc:
        with (
            tc.tile_pool(name="consts", bufs=1) as consts,
            tc.tile_pool(name="encp", bufs=24) as encp,
            tc.tile_pool(name="wlp", bufs=4) as wlp,
            tc.tile_pool(name="cwp", bufs=6) as cwp,
            tc.tile_pool(name="scrp", bufs=2) as scrp,
            tc.tile_pool(name="smp", bufs=4) as smp,
            tc.tile_pool(name="colp", bufs=12) as colp,
            tc.tile_pool(name="gp", bufs=1) as gp,
            tc.tile_pool(name="zps", bufs=2, space="PSUM") as zps,
            tc.tile_pool(name="tps", bufs=2, space="PSUM") as tps,
            tc.tile_pool(name="cps", bufs=1, space="PSUM") as cps,
        ):
            sf = consts.tile([P, w32], F32)
            nc.gpsimd.dma_start(out=sf, in_=sf_d[:])
            sb = consts.tile([P, KC * BPC], BF16)
            nc.gpsimd.dma_start(out=sb, in_=sb_d[:])
            ident = consts.tile([P, P], F32)
            make_identity(nc, ident)

            # ---------------- LSTM: z^T = W^T @ [x; h], feature-on-partition
            # PSUM allows one pending accumulation group per bank, so each
            # (kc, m) matmul is a closed start+stop group; the contraction
            # accumulates in SBUF via DVE adds.
            z_acc = gp.tile([P, 32 * BPC], F32, tag="zacc")
            for kc in range(KC):
                wt = wlp.tile([P, 4 * H], BF16)
                nc.scalar.dma_start(out=wt, in_=wl_d[kc])
                zpp = zps.tile([P, 32 * BPC], F32, tag="acc")
                for m in range(32):
                    nc.tensor.matmul(
                        zpp[:, m * BPC:(m + 1) * BPC],
                        wt[:, m * P:(m + 1) * P],
                        sb[:, kc * BPC:(kc + 1) * BPC],
                        start=True, stop=True,
                    )
                if kc == 0:
                    nc.vector.tensor_copy(z_acc, zpp)
                else:
                    nc.vector.tensor_add(z_acc, z_acc, zpp)
            gates = []
            for m in range(32):
                func = AF.Tanh if m // 8 == 2 else AF.Sigmoid
                g = gp.tile([P, BPC], F32, tag=f"g{m}")
                nc.scalar.activation(
                    out=g, in_=z_acc[:, m * BPC:(m + 1) * BPC], func=func,
                    bias=sf[:, OFF_BIAS + m:OFF_BIAS + m + 1])
                gates.append(g)
            hcout = gp.tile([P, 2 * HC * BPC], F32, tag="hcout")
            h_sb = []                                   # h_new chunks (f32)
            for hc in range(HC):
                i_g, f_g, g_g, o_g = (gates[hc], gates[8 + hc],
                                      gates[16 + hc], gates[24 + hc])
                t1 = gp.tile([P, BPC], F32, tag=f"t1{hc}")
                nc.vector.tensor_mul(
                    t1, f_g, sf[:, OFF_CT + hc * BPC:OFF_CT + (hc + 1) * BPC])
                t2 = gp.tile([P, BPC], F32, tag=f"t2{hc}")
                nc.vector.tensor_mul(t2, i_g, g_g)
                c_new = hcout[:, (HC + hc) * BPC:(HC + hc + 1) * BPC]
                nc.vector.tensor_add(c_new, t1, t2)
                t3 = gp.tile([P, BPC], F32, tag=f"t3{hc}")
                nc.scalar.activation(out=t3, in_=c_new, func=AF.Tanh)
                h_new = hcout[:, hc * BPC:(hc + 1) * BPC]
                nc.vector.tensor_mul(h_new, o_g, t3)
                h_sb.append(h_new)
            nc.sync.dma_start(out=hc_d[:], in_=hcout)

            # ---------------- attention per slot
            ctx_rows = []                               # (1, H) f32 per slot
            rsums = []
            for j in range(BPC):
                tj = T[j]
                ets = []
                sc = smp.tile([P, tj], F32, tag="sc")
                for t in range(tj):
                    et = encp.tile([P, H], F32, tag="enc")
                    nc.sync.dma_start(out=et, in_=enc_d[j][t * P:(t + 1) * P, :])
                    scr = scrp.tile([P, H], F32, tag="scr")
                    nc.vector.scalar_tensor_tensor(
                        out=scr, in0=et, scalar=1.0,
                        in1=sf[:, OFF_WENC:OFF_WENC + H],
                        op0=ALU.mult, op1=ALU.mult,
                        accum_out=sc[:, t:t + 1])
                    ets.append(et)
                scm = smp.tile([P, tj], F32, tag="scm")
                moff = OFF_MASK + sum(T[:j])
                nc.vector.tensor_add(scm, sc, sf[:, moff:moff + tj])
                mx = colp.tile([P, 1], F32, tag="mx")
                nc.vector.tensor_reduce(mx, scm, axis=mybir.AxisListType.X,
                                        op=ALU.max)
                amx = colp.tile([P, 1], F32, tag="amx")
                nc.gpsimd.partition_all_reduce(
                    amx, mx, channels=P, reduce_op=bass_isa.ReduceOp.max)
                namx = colp.tile([P, 1], F32, tag="namx")
                nc.scalar.mul(namx, amx, -1.0)
                ew = smp.tile([P, tj], F32, tag="ew")
                ssum = colp.tile([P, 1], F32, tag="ssum")
                nc.scalar.activation(out=ew, in_=scm, func=AF.Exp,
                                     bias=namx, accum_out=ssum)
                asum = colp.tile([P, 1], F32, tag="asum")
                nc.gpsimd.partition_all_reduce(
                    asum, ssum, channels=P, reduce_op=bass_isa.ReduceOp.add)
                rsum = colp.tile([P, 1], F32, tag=f"rsum{j}")
                nc.vector.reciprocal(rsum, asum)
                rsums.append(rsum)

                ctxp = cps.tile([1, H], F32, tag="ctx")
                for t in range(tj):
                    for half in range(2):
                        nc.tensor.matmul(
                            ctxp[:, half * 512:(half + 1) * 512],
                            ew[:, t:t + 1],
                            ets[t][:, half * 512:(half + 1) * 512],
                            start=(t == 0), stop=(t == tj - 1),
                        )
                ctx_row = gp.tile([1, H], F32, tag=f"ctx{j}")
                nc.scalar.activation(out=ctx_row, in_=ctxp, func=AF.Identity,
                                     scale=rsum[0:1, 0:1])
                ctx_rows.append(ctx_row)

            # ---------------- transpose context to feature-on-partition
            cat = []                                    # 16 chunks (P, BPC) f32
            for hc in range(HC):
                tp = tps.tile([P, BPC], F32, tag="tp")
                for j in range(BPC):
                    nc.tensor.transpose(
                        tp[:, j:j + 1],
                        ctx_rows[j][0:1, hc * P:(hc + 1) * P],
                        ident[0:1, 0:1])
                ct_sb = gp.tile([P, BPC], F32, tag=f"cat{hc}")
                nc.scalar.activation(out=ct_sb, in_=tp, func=AF.Identity)
                cat.append(ct_sb)
            cat.extend(h_sb)

            # ---------------- concat projection + tanh
            nh_acc = gp.tile([P, HC * BPC], F32, tag="nhacc")
            for kc in range(CW_KC):
                cwt = cwp.tile([P, H], F32, tag="cw")
                nc.scalar.dma_start(out=cwt, in_=cw_d[kc])
                nhp = zps.tile([P, HC * BPC], F32, tag="acc")
                for m in range(HC):
                    nc.tensor.matmul(
                        nhp[:, m * BPC:(m + 1) * BPC],
                        cwt[:, m * P:(m + 1) * P],
                        cat[kc], start=True, stop=True)
                if kc == 0:
                    nc.vector.tensor_copy(nh_acc, nhp)
                else:
                    nc.vector.tensor_add(nh_acc, nh_acc, nhp)
            nh = []
            for m in range(HC):
                nh_sb = gp.tile([P, BPC], F32, tag=f"nh{m}")
                nc.scalar.activation(
                    out=nh_sb, in_=nh_acc[:, m * BPC:(m + 1) * BPC], func=AF.Tanh,
                    bias=sf[:, OFF_CB + m:OFF_CB + m + 1])
                nh.append(nh_sb)

            # ---------------- output projection
            lp = zps.tile([VOCAB, BPC], F32, tag="lp")
            for kc in range(HC):
                nc.tensor.matmul(
                    lp, sf[:, OFF_OWT + kc * VOCAB:OFF_OWT + (kc + 1) * VOCAB],
                    nh[kc], start=(kc == 0), stop=(kc == HC - 1))
            lg_sb = gp.tile([VOCAB, BPC], F32, tag="lg")
            nc.scalar.activation(out=lg_sb, in_=lp, func=AF.Identity,
                                 bias=sf[0:VOCAB, OFF_OB:OFF_OB + 1])
            nc.sync.dma_start(out=lg_d[:], in_=lg_sb)

    nc.compile()
    return nc


_CACHE = {}


def _get_prog(T):
    T = tuple(T)
    if T not in _CACHE:
        _CACHE[T] = _build(T)
    return _CACHE[T]


def kernel(char, h_prev, c_prev, encoder_lens, encoder_hidden_states,
           embedding, w_ih, w_hh, b_ih, b_hh, attn_w, attn_b,
           concat_w, concat_b, out_w, out_b):
    char = np.asarray(char)
    h_prev = np.asarray(h_prev, dtype=np.float32)
    c_prev = np.asarray(c_prev, dtype=np.float32)
    lens = np.asarray(encoder_lens).astype(np.int64)
    enc = np.asarray(encoder_hidden_states, dtype=np.float32)
    embedding = np.asarray(embedding, dtype=np.float32)
    w_ih = np.asarray(w_ih, dtype=np.float32)
    w_hh = np.asarray(w_hh, dtype=np.float32)
    bias = (np.asarray(b_ih, dtype=np.float32)
            + np.asarray(b_hh, dtype=np.float32))
    attn_w = np.asarray(attn_w, dtype=np.float32)
    concat_w = np.asarray(concat_w, dtype=np.float32)
    concat_b = np.asarray(concat_b, dtype=np.float32)
    out_w = np.asarray(out_w, dtype=np.float32)
    out_b = np.asarray(out_b, dtype=np.float32)

    # sort batches by length; slot j of core k gets rank j*8+k
    order = np.argsort(-lens, kind="stable")
    T = tuple(max(1, math.ceil(int(lens[order[j * NCORES]]) / P))
              for j in range(BPC))
    prog = _get_prog(T)
    sum_t = sum(T)
    w32 = OFF_MASK + sum_t

    # ---- shared (core-independent) blobs
    wl = np.ascontiguousarray(
        np.concatenate([w_ih, w_hh], axis=1).T.reshape(KC, P, 4 * H)
    ).astype(BF_NP)
    cw = np.ascontiguousarray(concat_w.T.reshape(CW_KC, P, H))
    owt_cols = np.ascontiguousarray(
        out_w.T.reshape(HC, P, VOCAB).transpose(1, 0, 2).reshape(P, HC * VOCAB))

    sf_base = np.zeros((P, w32), np.float32)
    sf_base[:, OFF_WENC:OFF_WENC + H] = attn_w[:H]
    sf_base[:, OFF_BIAS:OFF_BIAS + 32] = bias.reshape(32, P).T
    sf_base[:, OFF_CB:OFF_CB + HC] = concat_b.reshape(HC, P).T
    sf_base[:VOCAB, OFF_OB] = out_b
    sf_base[:, OFF_OWT:OFF_OWT + HC * VOCAB] = owt_cols

    x_all = embedding[char]                      # (B, CHAR)
    xh_all = np.concatenate([x_all, h_prev[0]], axis=1)   # (B, CHAR+H)
    cpT = c_prev[0]                              # (B, H)

    sidx = np.arange(P)[:, None]                 # (128, 1)

    in_maps = []
    slot_batches = np.empty((NCORES, BPC), np.int64)
    for k in range(NCORES):
        sf = sf_base.copy()
        sb = np.zeros((P, KC * BPC), BF_NP)
        m = {}
        moff = OFF_MASK
        for j in range(BPC):
            b = int(order[j * NCORES + k])
            slot_batches[k, j] = b
            # additive mask (128, T_j)
            tj = T[j]
            pos = sidx + P * np.arange(tj)[None, :]
            sf[:, moff:moff + tj] = np.where(pos < lens[b], 0.0, NEG)
            moff += tj
            # c_prev^T chunks
            sf[:, OFF_CT + np.arange(HC) * BPC + j] = \
                cpT[b].reshape(HC, P).T[:, :]
            # xh^T chunks
            sb[:, np.arange(KC) * BPC + j] = \
                xh_all[b].reshape(KC, P).T.astype(BF_NP)
            m[f"enc{j}"] = enc[b, :tj * P, :]
        m["sf"] = sf
        m["sb"] = sb
        m["wl"] = wl
        m["cw"] = cw
        in_maps.append(m)

    res = run_bass_kernel_spmd(prog, in_maps, list(range(NCORES)))

    logits = np.zeros((B, VOCAB), np.float32)
    h_new = np.zeros((B, H), np.float32)
    c_new = np.zeros((B, H), np.float32)
    for k in range(NCORES):
        r = res.results[k]
        lg = r["lg"]                              # (VOCAB, BPC)
        hc = r["hc"]                              # (P, 2*HC*BPC)
        for j in range(BPC):
            b = slot_batches[k, j]
            logits[b] = lg[:, j]
            h_new[b] = hc[:, np.arange(HC) * BPC + j].T.reshape(H)
            c_new[b] = hc[:, (HC + np.arange(HC)) * BPC + j].T.reshape(H)
    return logits, h_new[None], c_new[None]
